# revision 11
# baseline (speedup 1.0000x reference)
"""LRU single-step kernel for 8x TRN2 NeuronCores (Bass/Tile), bf16 I/O.

Math (per batch row b, hidden h):
  out_re[b,h] = lam_re[h]*h_re[b,h] - lam_im[h]*h_im[b,h] + (x @ (scale*B_real).T)[b,h]
  out_im[b,h] = lam_im[h]*h_re[b,h] + lam_re[h]*h_im[b,h] + (x @ (scale*B_img ).T)[b,h]

Strategy: data-parallel over the batch axis (8 shards of 32768 rows). On each
core, everything is computed in a transposed layout (hidden on partitions,
batch on the free axis) so that the Lambda elementwise terms become diagonal-
weight matmuls accumulating into the same PSUM tile as the input projection:

  psum_re[h,b] = W_re[i,h].T @ x_t[i,b] + diag(lam_re) @ hre_t[h,b] + diag(-lam_im) @ him_t[h,b]

All HBM traffic is bf16: this halves DMA bytes (the kernel's roofline) and
runs the PE at 1 cycle/row instead of fp32's 4. PSUM accumulates in fp32;
the PSUM->SBUF evacuation copies cast fp32->bf16. Outputs are upcast to
fp32 on the host. Quantization rel-l2 error ~2.9e-3 vs the 2e-2 gate.

DMA layout: the host packs x^T, h_re^T, h_im^T into ONE dram tensor shaped
(128, OUTER, 5, COLS) so each iteration's entire input slab is a single
2.5 MiB DMA with 20 KB-per-partition descriptors (real-HW DMA efficiency
rises steeply with transfer size). Outputs similarly leave as one
(128, 4, COLS) slab per iteration = one 2 MiB DMA.

Engine assignment: loads on Pool (SWDGE), the merged store on ACT (HWDGE) —
store waits therefore never head-of-line-block the load stream, which kept
the DMA engines 97%+ busy in the timeline model.

PE Matmult instructions only have one sync-wait slot in codegen, so waits
are absorbed before real matmuls run:
  - a per-iteration 1x1 "lane absorber" matmul reads the freshly-DMA'd in
    tile (writing a persistent scratch PSUM tile), so it carries the DMA
    wait and advances the PE's observed clock;
  - PSUM tiles are allocated once and reused manually (no pool recycling),
    so no TileRelease edges exist on PSUM: the first matmul of a group
    carries only the WAR wait on the previous use's PSUM->SBUF copy.
"""

import ml_dtypes
import numpy as np

import concourse.bass as bass
import concourse.mybir as mybir
from concourse.tile import TileContext
from concourse.bass_utils import run_bass_kernel_spmd

B_SZ, IN_DIM, HID = 262144, 128, 256
N_CORES = 8
S = B_SZ // N_CORES  # 32768 rows per core
P = 128
HCHUNKS = HID // P  # 2
COLS = 2048          # batch columns per outer iteration
OUTER = S // COLS    # 16
MMF = 512            # matmul free dim (one fp32 PSUM bank)
NBLK = COLS // MMF   # 4

# in slab chunks (dim of size 5): [x, hre_c0, hre_c1, him_c0, him_c1]
# out slab chunks (dim of size 4): [ore_c0, ore_c1, oim_c0, oim_c1]

# consts layout (one (128, 1280) bf16 tensor):
#   [:, 0:256]     w_re  = (scale*B_real).T
#   [:, 256:512]   w_im  = (scale*B_img).T
#   [:, 512:768]   diag(lam_re)  chunks 0,1
#   [:, 768:1024]  diag(lam_im)  chunks 0,1
#   [:, 1024:1280] diag(-lam_im) chunks 0,1
CONST_COLS = 1280

F32 = mybir.dt.float32
BF16 = mybir.dt.bfloat16
NP_BF16 = ml_dtypes.bfloat16

_cache = {}

# Stashed BassKernelResults from the most recent run (for test harnesses).
LAST_RESULTS = None


def _build():
    if "nc" in _cache:
        return _cache["nc"]

    nc = bass.Bass(trn_type="TRN2")

    in_t = nc.dram_tensor("in_t", (P, OUTER, 5, COLS), BF16, kind="ExternalInput")
    consts = nc.dram_tensor("consts", (P, CONST_COLS), BF16, kind="ExternalInput")
    out_t = nc.dram_tensor("out_t", (P, OUTER, 4, COLS), BF16, kind="ExternalOutput")

    with TileContext(nc) as tc:
        with (
            tc.tile_pool(name="cpool", bufs=1) as cpool,
            tc.tile_pool(name="inp", bufs=5) as inp,
            tc.tile_pool(name="outp", bufs=4) as outp,
            tc.tile_pool(name="psum", bufs=1, space="PSUM") as psum,
        ):
            csb = cpool.tile([P, CONST_COLS], BF16)
            nc.sync.dma_start(csb[:], consts[:, :])
            # 7 persistent data PSUM tiles + 1 scratch; allocated once so no
            # TileRelease/realloc wait sets ever form on PSUM.
            ps_tiles = [psum.tile([P, MMF], F32, tag=f"ps{i}", name=f"ps{i}")
                        for i in range(7)]
            scratch = psum.tile([P, 8], F32, tag="scratch")
            _cache["ps_idx"] = 0

            def lane_absorb(tile_ap):
                # 1x1 matmul reading the freshly-DMA'd tile: carries exactly
                # one DMA-lane wait, advancing the PE's observed clock so the
                # real matmuls don't re-wait on that lane.
                nc.tensor.matmul(scratch[0:1, 0:1], tile_ap, tile_ap,
                                 start=True, stop=True, skip_group_check=True)

            w_re_sb = csb[:, 0:HID]
            w_im_sb = csb[:, HID:2 * HID]

            def dre_c(c):
                return csb[:, 2 * HID + c * P: 2 * HID + (c + 1) * P]

            def dim_c(c):
                return csb[:, 3 * HID + c * P: 3 * HID + (c + 1) * P]

            def dimn_c(c):
                return csb[:, 4 * HID + c * P: 4 * HID + (c + 1) * P]

            lane_absorb(csb[0:1, 0:1])

            def block(it, ot, c, b):
                wre_c = w_re_sb[:, c * P:(c + 1) * P]
                wim_c = w_im_sb[:, c * P:(c + 1) * P]
                bs = slice(b * MMF, (b + 1) * MMF)
                xs = it[:, 0, bs]
                hres = it[:, 1 + c, bs]
                hims = it[:, 3 + c, bs]

                ps_re = ps_tiles[_cache["ps_idx"] % 7]
                _cache["ps_idx"] += 1
                nc.tensor.matmul(ps_re[:], wre_c, xs, start=True, stop=False)
                nc.tensor.matmul(ps_re[:], dre_c(c), hres, start=False, stop=False)
                nc.tensor.matmul(ps_re[:], dimn_c(c), hims, start=False, stop=True)

                ps_im = ps_tiles[_cache["ps_idx"] % 7]
                _cache["ps_idx"] += 1
                nc.tensor.matmul(ps_im[:], wim_c, xs, start=True, stop=False)
                nc.tensor.matmul(ps_im[:], dim_c(c), hres, start=False, stop=False)
                nc.tensor.matmul(ps_im[:], dre_c(c), hims, start=False, stop=True)

                # ore chunks via ACT, oim chunks via DVE (parallel
                # PSUM reads from different banks).
                nc.scalar.copy(ot[:, c, bs], ps_re[:])
                nc.vector.tensor_copy(ot[:, 2 + c, bs], ps_im[:])

            for o in range(OUTER):
                it = inp.tile([P, 5, COLS], BF16)
                # Loads on SP's HWDGE ring: SP is otherwise idle, so load
                # issue never waits behind store data-dependencies, and the
                # first load skips Pool's SWDGE-prologue latency.
                nc.sync.dma_start(it[:], in_t[:, o, :, :])
                lane_absorb(it[0:1, 0, 0:1])

                ot = outp.tile([P, 4, COLS], BF16)

                if o < OUTER - 2:
                    for c in range(HCHUNKS):
                        for b in range(NBLK):
                            block(it, ot, c, b)
                    # Merged store from ACT (HWDGE): queues behind ACT's own
                    # copies; waits only on DVE's last oim copy. Never
                    # blocks the load stream.
                    nc.scalar.dma_start(out_t[:, o, :, :], ot[:])
                else:
                    # Last iterations: b-major order with per-stripe stores,
                    # so the pipeline-drain tail is one stripe instead of a
                    # whole 2048-col slab. The very last slab uses 2-block
                    # stripes to shrink the final drain further.
                    nstripe = 1 if o == OUTER - 1 else 2
                    for b0 in range(0, NBLK, nstripe):
                        for b in range(b0, b0 + nstripe):
                            for c in range(HCHUNKS):
                                block(it, ot, c, b)
                        bs = slice(b0 * MMF, (b0 + nstripe) * MMF)
                        nc.scalar.dma_start(out_t[:, o, :, bs], ot[:, :, bs])

    _split_multiwaits(nc)
    _cache["nc"] = nc
    return nc


def _split_multiwaits(nc):
    """walrus codegen allows exactly one semaphore wait per instruction.
    Move all-but-one wait of every multi-wait instruction onto single-wait
    NOP instructions spliced immediately before it on the same engine
    (engines execute their stream in order, so semantics are unchanged)."""
    k = 0
    for bb in nc.m.functions[0].blocks:
        new_list = []
        for ins in bb.instructions:
            si = ins.sync_info
            if si is not None and si.on_wait and len(si.on_wait) > 1:
                for w in si.on_wait[:-1]:
                    nop = mybir.InstNoOp(
                        name=f"WN-{k}", engine=ins.engine,
                        sync_info=mybir.SyncInfo(on_wait=[w], on_update=[]),
                    )
                    k += 1
                    new_list.append(nop)
                si.on_wait = [si.on_wait[-1]]
            new_list.append(ins)
        bb.instructions[:] = new_list


def kernel(inputs, h_re, h_im, nu_log, theta_log, B_real, B_img, gamma_log):
    global LAST_RESULTS
    inputs = np.asarray(inputs, dtype=np.float32)
    h_re = np.asarray(h_re, dtype=np.float32)
    h_im = np.asarray(h_im, dtype=np.float32)
    nu_log = np.asarray(nu_log, dtype=np.float32)
    theta_log = np.asarray(theta_log, dtype=np.float32)
    B_real = np.asarray(B_real, dtype=np.float32)
    B_img = np.asarray(B_img, dtype=np.float32)
    gamma_log = np.asarray(gamma_log, dtype=np.float32)

    # Tiny parameter math on host (matches the f32 reference computation).
    mag = np.exp(-np.exp(nu_log))          # (1, H)
    theta = np.exp(theta_log)              # (1, H)
    lam_re = (mag * np.cos(theta))[0]      # (H,)
    lam_im = (mag * np.sin(theta))[0]      # (H,)
    scale = np.exp(gamma_log).T            # (H, 1)
    w_re = (scale * B_real).T              # (IN_DIM, H)
    w_im = (scale * B_img).T               # (IN_DIM, H)

    consts = np.zeros((P, CONST_COLS), np.float32)
    consts[:, 0:HID] = w_re
    consts[:, HID:2 * HID] = w_im
    idx = np.arange(P)
    for c in range(HCHUNKS):
        lr = lam_re[c * P:(c + 1) * P]
        li = lam_im[c * P:(c + 1) * P]
        consts[idx, 2 * HID + c * P + idx] = lr
        consts[idx, 3 * HID + c * P + idx] = li
        consts[idx, 4 * HID + c * P + idx] = -li
    consts = consts.astype(NP_BF16)

    x_bf = inputs.astype(NP_BF16)
    hre_bf = h_re.astype(NP_BF16)
    him_bf = h_im.astype(NP_BF16)

    in_maps = []
    for core in range(N_CORES):
        sl = slice(core * S, (core + 1) * S)
        # (S, D).T -> (D, S) -> (D, OUTER, COLS)
        xT = x_bf[sl].T.reshape(IN_DIM, OUTER, COLS)
        hreT = hre_bf[sl].T.reshape(HID, OUTER, COLS)
        himT = him_bf[sl].T.reshape(HID, OUTER, COLS)
        slab = np.empty((P, OUTER, 5, COLS), NP_BF16)
        slab[:, :, 0, :] = xT
        slab[:, :, 1, :] = hreT[0:P]
        slab[:, :, 2, :] = hreT[P:2 * P]
        slab[:, :, 3, :] = himT[0:P]
        slab[:, :, 4, :] = himT[P:2 * P]
        in_maps.append({"in_t": slab, "consts": consts})

    nc = _build()
    res = run_bass_kernel_spmd(nc, in_maps, core_ids=list(range(N_CORES)))
    LAST_RESULTS = res

    out = np.empty((2, B_SZ, HID), np.float32)
    for core in range(N_CORES):
        sl = slice(core * S, (core + 1) * S)
        ob = res.results[core]["out_t"].astype(np.float32)  # (P, OUTER, 4, COLS)
        # chunk k of hidden for o_re is ob[:, :, k]: out[0, sl] rows map as
        # o_re[b, c*P + p] = ob[p, o, c, col] with b = o*COLS + col
        for c in range(HCHUNKS):
            out[0, sl, c * P:(c + 1) * P] = (
                ob[:, :, c, :].reshape(P, S).T)
            out[1, sl, c * P:(c + 1) * P] = (
                ob[:, :, 2 + c, :].reshape(P, S).T)
    return out


# revision 14
# speedup vs baseline: 1.1964x; 1.1964x over previous
"""LRU single-step kernel for 8x TRN2 NeuronCores (Bass/Tile).

bf16 x/weights/outputs, fp8(e4m3) h_re/h_im.

Math (per batch row b, hidden h):
  out_re[b,h] = lam_re[h]*h_re[b,h] - lam_im[h]*h_im[b,h] + (x @ (scale*B_real).T)[b,h]
  out_im[b,h] = lam_im[h]*h_re[b,h] + lam_re[h]*h_im[b,h] + (x @ (scale*B_img ).T)[b,h]

Strategy: data-parallel over the batch axis (8 shards of 32768 rows). On each
core, everything is computed in a transposed layout (hidden on partitions,
batch on the free axis) so that the Lambda elementwise terms become diagonal-
weight matmuls accumulating into the same PSUM tile as the input projection:

  psum_re[h,b] = W_re[i,h].T @ x_t[i,b] + diag(lam_re) @ hre_t[h,b] + diag(-lam_im) @ him_t[h,b]

Precision: the kernel is HBM-bandwidth-bound, so input/output bits are the
roofline. x, weights and outputs travel as bf16; h_re/h_im travel as fp8
e4m3 (halving the largest input stream). The lam*h contribution is only
~5% of output variance, so fp8's ~3.6% element error contributes ~6e-3
rel-l2 total — well inside the 2e-2 gate (measured 6.6e-3 on the actual
problem inputs). The diag(lam) weights stay bf16 (mixed-dtype matmul:
non-fp32 dtypes may differ between stationary and moving operands); PSUM
accumulates fp32; PSUM->SBUF copies cast to bf16; host upcasts to fp32.

DMA layout: per iteration one bf16 x slab (128, COLS) and one fp8 h slab
(128, 4, COLS) [hre_c0, hre_c1, him_c0, him_c1] are loaded as single DMAs
with multi-KB per-partition descriptors; outputs leave as one bf16
(128, 4, COLS) slab [ore_c0, ore_c1, oim_c0, oim_c1] per iteration.

Engine assignment: loads on SP's HWDGE ring (SP is otherwise idle, so load
issue never waits behind store data-dependencies); the merged store on ACT
(queues in-order behind ACT's own copies). This keeps the DMA engines >95%
busy in the timeline model. The last two iterations run b-major with
per-stripe stores to shrink the pipeline-drain tail.

PE Matmult instructions only have one sync-wait slot in codegen, so waits
are absorbed before real matmuls run:
  - per-iteration 1x1 "lane absorber" matmuls read the freshly-DMA'd
    tiles (writing a persistent scratch PSUM tile), so they carry the DMA
    waits and advance the PE's observed clock;
  - PSUM tiles are allocated once and reused manually (no pool recycling),
    so no TileRelease edges exist on PSUM: the first matmul of a group
    carries only the WAR wait on the previous use's PSUM->SBUF copy.
"""

import ml_dtypes
import numpy as np

import concourse.bass as bass
import concourse.mybir as mybir
from concourse.tile import TileContext
from concourse.bass_utils import run_bass_kernel_spmd

B_SZ, IN_DIM, HID = 262144, 128, 256
N_CORES = 8
S = B_SZ // N_CORES  # 32768 rows per core
P = 128
HCHUNKS = HID // P  # 2
COLS = 2048          # max batch columns per outer iteration
MMF = 512            # matmul free dim (one fp32 PSUM bank)
# Tapered iteration widths: small slabs at the start (PE begins compute
# ~5us earlier) and at the end (small pipeline-drain tail).
WIDTHS = [512, 1536] + [2048] * 14 + [1024, 512, 512]
assert sum(WIDTHS) == S and all(w % MMF == 0 and w <= COLS for w in WIDTHS)

# h slab chunks (dim of size 4): [hre_c0, hre_c1, him_c0, him_c1]
# out slab chunks (dim of size 4): [ore_c0, ore_c1, oim_c0, oim_c1]

# consts layout (one (128, 1280) bf16 tensor):
#   [:, 0:256]     w_re  = (scale*B_real).T
#   [:, 256:512]   w_im  = (scale*B_img).T
#   [:, 512:768]   diag(lam_re)  chunks 0,1
#   [:, 768:1024]  diag(lam_im)  chunks 0,1
#   [:, 1024:1280] diag(-lam_im) chunks 0,1
CONST_COLS = 1280

F32 = mybir.dt.float32
BF16 = mybir.dt.bfloat16
FP8 = mybir.dt.float8e4
NP_BF16 = ml_dtypes.bfloat16
NP_FP8 = mybir.dt.np(mybir.dt.float8e4)

_cache = {}

# Stashed BassKernelResults from the most recent run (for test harnesses).
LAST_RESULTS = None


def _build():
    if "nc" in _cache:
        return _cache["nc"]

    nc = bass.Bass(trn_type="TRN2")

    in_x = nc.dram_tensor("in_x", (P, S), BF16, kind="ExternalInput")
    in_h = nc.dram_tensor("in_h", (P, 4, S), FP8, kind="ExternalInput")
    consts = nc.dram_tensor("consts", (P, CONST_COLS), BF16, kind="ExternalInput")
    out_t = nc.dram_tensor("out_t", (P, 4, S), BF16, kind="ExternalOutput")

    with TileContext(nc) as tc:
        with (
            tc.tile_pool(name="cpool", bufs=1) as cpool,
            tc.tile_pool(name="xin", bufs=5) as xin,
            tc.tile_pool(name="hin", bufs=5) as hin,
            tc.tile_pool(name="outp", bufs=4) as outp,
            tc.tile_pool(name="psum", bufs=1, space="PSUM") as psum,
        ):
            csb = cpool.tile([P, CONST_COLS], BF16)
            nc.sync.dma_start(csb[:], consts[:, :])
            # 7 persistent data PSUM tiles + 1 scratch; allocated once so no
            # TileRelease/realloc wait sets ever form on PSUM.
            ps_tiles = [psum.tile([P, MMF], F32, tag=f"ps{i}", name=f"ps{i}")
                        for i in range(7)]
            scratch = psum.tile([P, 8], F32, tag="scratch")
            _cache["ps_idx"] = 0

            def lane_absorb(tile_ap):
                # 1x1 matmul reading the freshly-DMA'd tile: carries exactly
                # one DMA-lane wait, advancing the PE's observed clock so the
                # real matmuls don't re-wait on that lane.
                nc.tensor.matmul(scratch[0:1, 0:1], tile_ap, tile_ap,
                                 start=True, stop=True, skip_group_check=True)

            w_re_sb = csb[:, 0:HID]
            w_im_sb = csb[:, HID:2 * HID]

            def dre_c(c):
                return csb[:, 2 * HID + c * P: 2 * HID + (c + 1) * P]

            def dim_c(c):
                return csb[:, 3 * HID + c * P: 3 * HID + (c + 1) * P]

            def dimn_c(c):
                return csb[:, 4 * HID + c * P: 4 * HID + (c + 1) * P]

            lane_absorb(csb[0:1, 0:1])

            def block(xt, ht, ot, c, b):
                wre_c = w_re_sb[:, c * P:(c + 1) * P]
                wim_c = w_im_sb[:, c * P:(c + 1) * P]
                bs = slice(b * MMF, (b + 1) * MMF)
                xs = xt[:, bs]
                hres = ht[:, c, bs]
                hims = ht[:, 2 + c, bs]

                ps_re = ps_tiles[_cache["ps_idx"] % 7]
                _cache["ps_idx"] += 1
                nc.tensor.matmul(ps_re[:], wre_c, xs, start=True, stop=False)
                nc.tensor.matmul(ps_re[:], dre_c(c), hres, start=False, stop=False)
                nc.tensor.matmul(ps_re[:], dimn_c(c), hims, start=False, stop=True)

                ps_im = ps_tiles[_cache["ps_idx"] % 7]
                _cache["ps_idx"] += 1
                nc.tensor.matmul(ps_im[:], wim_c, xs, start=True, stop=False)
                nc.tensor.matmul(ps_im[:], dim_c(c), hres, start=False, stop=False)
                nc.tensor.matmul(ps_im[:], dre_c(c), hims, start=False, stop=True)

                # ore chunks via ACT, oim chunks via DVE (parallel
                # PSUM reads from different banks).
                nc.scalar.copy(ot[:, c, bs], ps_re[:])
                nc.vector.tensor_copy(ot[:, 2 + c, bs], ps_im[:])

            pos = 0
            for w in WIDTHS:
                sl = slice(pos, pos + w)
                pos += w
                nblk = w // MMF
                xt = xin.tile([P, COLS], BF16)
                ht = hin.tile([P, 4, COLS], FP8)
                nc.sync.dma_start(xt[:, 0:w], in_x[:, sl])
                nc.sync.dma_start(ht[:, :, 0:w], in_h[:, :, sl])
                lane_absorb(xt[0:1, 0:1])
                lane_absorb(ht[0:1, 0, 0:1])

                ot = outp.tile([P, 4, COLS], BF16)

                for c in range(HCHUNKS):
                    for b in range(nblk):
                        block(xt, ht, ot, c, b)
                # Merged store from ACT (HWDGE): queues behind ACT's own
                # copies; waits only on DVE's last oim copy. Never
                # blocks the load stream.
                nc.scalar.dma_start(out_t[:, :, sl], ot[:, :, 0:w])

    _split_multiwaits(nc)
    _cache["nc"] = nc
    return nc


def _split_multiwaits(nc):
    """walrus codegen allows exactly one semaphore wait per instruction.
    Move all-but-one wait of every multi-wait instruction onto single-wait
    NOP instructions spliced immediately before it on the same engine
    (engines execute their stream in order, so semantics are unchanged)."""
    k = 0
    for bb in nc.m.functions[0].blocks:
        new_list = []
        for ins in bb.instructions:
            si = ins.sync_info
            if si is not None and si.on_wait and len(si.on_wait) > 1:
                for w in si.on_wait[:-1]:
                    nop = mybir.InstNoOp(
                        name=f"WN-{k}", engine=ins.engine,
                        sync_info=mybir.SyncInfo(on_wait=[w], on_update=[]),
                    )
                    k += 1
                    new_list.append(nop)
                si.on_wait = [si.on_wait[-1]]
            new_list.append(ins)
        bb.instructions[:] = new_list


def kernel(inputs, h_re, h_im, nu_log, theta_log, B_real, B_img, gamma_log):
    global LAST_RESULTS
    inputs = np.asarray(inputs, dtype=np.float32)
    h_re = np.asarray(h_re, dtype=np.float32)
    h_im = np.asarray(h_im, dtype=np.float32)
    nu_log = np.asarray(nu_log, dtype=np.float32)
    theta_log = np.asarray(theta_log, dtype=np.float32)
    B_real = np.asarray(B_real, dtype=np.float32)
    B_img = np.asarray(B_img, dtype=np.float32)
    gamma_log = np.asarray(gamma_log, dtype=np.float32)

    # Tiny parameter math on host (matches the f32 reference computation).
    mag = np.exp(-np.exp(nu_log))          # (1, H)
    theta = np.exp(theta_log)              # (1, H)
    lam_re = (mag * np.cos(theta))[0]      # (H,)
    lam_im = (mag * np.sin(theta))[0]      # (H,)
    scale = np.exp(gamma_log).T            # (H, 1)
    w_re = (scale * B_real).T              # (IN_DIM, H)
    w_im = (scale * B_img).T               # (IN_DIM, H)

    consts = np.zeros((P, CONST_COLS), np.float32)
    consts[:, 0:HID] = w_re
    consts[:, HID:2 * HID] = w_im
    idx = np.arange(P)
    for c in range(HCHUNKS):
        lr = lam_re[c * P:(c + 1) * P]
        li = lam_im[c * P:(c + 1) * P]
        consts[idx, 2 * HID + c * P + idx] = lr
        consts[idx, 3 * HID + c * P + idx] = li
        consts[idx, 4 * HID + c * P + idx] = -li
    consts = consts.astype(NP_BF16)

    x_bf = inputs.astype(NP_BF16)
    hre_q = h_re.astype(NP_FP8)
    him_q = h_im.astype(NP_FP8)

    in_maps = []
    for core in range(N_CORES):
        sl = slice(core * S, (core + 1) * S)
        xT = np.ascontiguousarray(x_bf[sl].T)          # (128, S)
        hreT = hre_q[sl].T                             # (256, S)
        himT = him_q[sl].T
        hslab = np.empty((P, 4, S), NP_FP8)
        hslab[:, 0, :] = hreT[0:P]
        hslab[:, 1, :] = hreT[P:2 * P]
        hslab[:, 2, :] = himT[0:P]
        hslab[:, 3, :] = himT[P:2 * P]
        in_maps.append({"in_x": xT, "in_h": hslab, "consts": consts})

    nc = _build()
    res = run_bass_kernel_spmd(nc, in_maps, core_ids=list(range(N_CORES)))
    LAST_RESULTS = res

    out = np.empty((2, B_SZ, HID), np.float32)
    for core in range(N_CORES):
        sl = slice(core * S, (core + 1) * S)
        ob = res.results[core]["out_t"].astype(np.float32)  # (P, 4, S)
        for c in range(HCHUNKS):
            out[0, sl, c * P:(c + 1) * P] = ob[:, c, :].T
            out[1, sl, c * P:(c + 1) * P] = ob[:, 2 + c, :].T
    return out


# revision 15
# speedup vs baseline: 1.2902x; 1.0784x over previous
"""LRU single-step kernel for 8x TRN2 NeuronCores (Bass/Tile).

bf16 x/weights/outputs, fp8(e4m3) h_re/h_im.

Math (per batch row b, hidden h):
  out_re[b,h] = lam_re[h]*h_re[b,h] - lam_im[h]*h_im[b,h] + (x @ (scale*B_real).T)[b,h]
  out_im[b,h] = lam_im[h]*h_re[b,h] + lam_re[h]*h_im[b,h] + (x @ (scale*B_img ).T)[b,h]

Strategy: data-parallel over the batch axis (8 shards of 32768 rows). On each
core, everything is computed in a transposed layout (hidden on partitions,
batch on the free axis) so that the Lambda elementwise terms become diagonal-
weight matmuls accumulating into the same PSUM tile as the input projection:

  psum_re[h,b] = W_re[i,h].T @ x_t[i,b] + diag(lam_re) @ hre_t[h,b] + diag(-lam_im) @ him_t[h,b]

Precision: the kernel is HBM-bandwidth-bound, so input/output bits are the
roofline. x, weights and outputs travel as bf16; h_re/h_im travel as fp8
e4m3 (halving the largest input stream). The lam*h contribution is only
~5% of output variance, so fp8's ~3.6% element error contributes ~6e-3
rel-l2 total — well inside the 2e-2 gate (measured 6.6e-3 on the actual
problem inputs). The diag(lam) weights stay bf16 (mixed-dtype matmul:
non-fp32 dtypes may differ between stationary and moving operands); PSUM
accumulates fp32; PSUM->SBUF copies cast to bf16; host upcasts to fp32.

DMA layout: per iteration one bf16 x slab (128, COLS) and one fp8 h slab
(128, 4, COLS) [hre_c0, hre_c1, him_c0, him_c1] are loaded as single DMAs
with multi-KB per-partition descriptors; outputs leave as one bf16
(128, 4, COLS) slab [ore_c0, ore_c1, oim_c0, oim_c1] per iteration.

Engine assignment: loads on SP's HWDGE ring (SP is otherwise idle, so load
issue never waits behind store data-dependencies); the merged store on ACT
(queues in-order behind ACT's own copies). This keeps the DMA engines >95%
busy in the timeline model. The last two iterations run b-major with
per-stripe stores to shrink the pipeline-drain tail.

PE Matmult instructions only have one sync-wait slot in codegen, so waits
are absorbed before real matmuls run:
  - per-iteration 1x1 "lane absorber" matmuls read the freshly-DMA'd
    tiles (writing a persistent scratch PSUM tile), so they carry the DMA
    waits and advance the PE's observed clock;
  - PSUM tiles are allocated once and reused manually (no pool recycling),
    so no TileRelease edges exist on PSUM: the first matmul of a group
    carries only the WAR wait on the previous use's PSUM->SBUF copy.
"""

import ml_dtypes
import numpy as np

import concourse.bass as bass
import concourse.mybir as mybir
from concourse.tile import TileContext
from concourse.bass_utils import run_bass_kernel_spmd

B_SZ, IN_DIM, HID = 262144, 128, 256
N_CORES = 8
S = B_SZ // N_CORES  # 32768 rows per core
P = 128
HCHUNKS = HID // P  # 2
COLS = 2048          # max batch columns per outer iteration
MMF = 512            # matmul free dim (one fp32 PSUM bank)
# Tapered iteration widths: small slabs at the start (PE begins compute
# ~5us earlier) and at the end (small pipeline-drain tail).
WIDTHS = [512, 1536] + [2048] * 14 + [1024, 512, 512]
assert sum(WIDTHS) == S and all(w % MMF == 0 and w <= COLS for w in WIDTHS)

# h slab chunks (dim of size 4): [hre_c0, hre_c1, him_c0, him_c1]
# out slab chunks (dim of size 4): [ore_c0, ore_c1, oim_c0, oim_c1]

# consts layout (one (128, 1280) bf16 tensor):
#   [:, 0:256]     w_re  = (scale*B_real).T
#   [:, 256:512]   w_im  = (scale*B_img).T
#   [:, 512:768]   diag(lam_re)  chunks 0,1
#   [:, 768:1024]  diag(lam_im)  chunks 0,1
#   [:, 1024:1280] diag(-lam_im) chunks 0,1
CONST_COLS = 1280

F32 = mybir.dt.float32
BF16 = mybir.dt.bfloat16
FP8 = mybir.dt.float8e4
NP_BF16 = ml_dtypes.bfloat16
NP_FP8 = mybir.dt.np(mybir.dt.float8e4)

_cache = {}

# Stashed BassKernelResults from the most recent run (for test harnesses).
LAST_RESULTS = None


def _build():
    if "nc" in _cache:
        return _cache["nc"]

    nc = bass.Bass(trn_type="TRN2")

    in_x = nc.dram_tensor("in_x", (P, S), BF16, kind="ExternalInput")
    in_h = nc.dram_tensor("in_h", (P, 4, S), FP8, kind="ExternalInput")
    consts = nc.dram_tensor("consts", (P, CONST_COLS), BF16, kind="ExternalInput")
    out_t = nc.dram_tensor("out_t", (P, 4, S), BF16, kind="ExternalOutput")

    with TileContext(nc) as tc:
        with (
            tc.tile_pool(name="cpool", bufs=1) as cpool,
            tc.tile_pool(name="xin", bufs=5) as xin,
            tc.tile_pool(name="hin", bufs=5) as hin,
            tc.tile_pool(name="outp", bufs=4) as outp,
            tc.tile_pool(name="psum", bufs=1, space="PSUM") as psum,
        ):
            csb = cpool.tile([P, CONST_COLS], BF16)
            nc.sync.dma_start(csb[:], consts[:, :])
            # 7 persistent data PSUM tiles + 1 scratch; allocated once so no
            # TileRelease/realloc wait sets ever form on PSUM.
            ps_tiles = [psum.tile([P, MMF], F32, tag=f"ps{i}", name=f"ps{i}")
                        for i in range(7)]
            scratch = psum.tile([P, MMF], F32, tag="scratch")
            _cache["ps_idx"] = 0

            # PE pre-warm: while the first loads are in flight the PE would
            # sit cold (HAM keeps it at the low/mid clock until ~3us of
            # sustained activity). Keep it busy on junk matmuls over an
            # uninitialized SBUF tile (results land in the scratch PSUM
            # bank and are never read) so the real matmuls start at the
            # full 2.4 GHz clock.
            dummy = cpool.tile([P, 256], BF16, tag="prewarm")
            nc.gpsimd.memset(dummy[:], 0.0)
            for _ in range(16):
                nc.tensor.matmul(scratch[0:1, 0:256], dummy[:, 0:1],
                                 dummy[:, 0:256], start=True, stop=True,
                                 skip_group_check=True)

            def lane_absorb(tile_ap):
                # 1x1 matmul reading the freshly-DMA'd tile: carries exactly
                # one DMA-lane wait, advancing the PE's observed clock so the
                # real matmuls don't re-wait on that lane.
                nc.tensor.matmul(scratch[0:1, 0:1], tile_ap, tile_ap,
                                 start=True, stop=True, skip_group_check=True)

            w_re_sb = csb[:, 0:HID]
            w_im_sb = csb[:, HID:2 * HID]

            def dre_c(c):
                return csb[:, 2 * HID + c * P: 2 * HID + (c + 1) * P]

            def dim_c(c):
                return csb[:, 3 * HID + c * P: 3 * HID + (c + 1) * P]

            def dimn_c(c):
                return csb[:, 4 * HID + c * P: 4 * HID + (c + 1) * P]

            lane_absorb(csb[0:1, 0:1])

            def block(xt, ht, ot, c, b):
                wre_c = w_re_sb[:, c * P:(c + 1) * P]
                wim_c = w_im_sb[:, c * P:(c + 1) * P]
                bs = slice(b * MMF, (b + 1) * MMF)
                xs = xt[:, bs]
                hres = ht[:, c, bs]
                hims = ht[:, 2 + c, bs]

                ps_re = ps_tiles[_cache["ps_idx"] % 7]
                _cache["ps_idx"] += 1
                nc.tensor.matmul(ps_re[:], wre_c, xs, start=True, stop=False)
                nc.tensor.matmul(ps_re[:], dre_c(c), hres, start=False, stop=False)
                nc.tensor.matmul(ps_re[:], dimn_c(c), hims, start=False, stop=True)

                ps_im = ps_tiles[_cache["ps_idx"] % 7]
                _cache["ps_idx"] += 1
                nc.tensor.matmul(ps_im[:], wim_c, xs, start=True, stop=False)
                nc.tensor.matmul(ps_im[:], dim_c(c), hres, start=False, stop=False)
                nc.tensor.matmul(ps_im[:], dre_c(c), hims, start=False, stop=True)

                # ore chunks via ACT, oim chunks via DVE (parallel
                # PSUM reads from different banks).
                nc.scalar.copy(ot[:, c, bs], ps_re[:])
                nc.vector.tensor_copy(ot[:, 2 + c, bs], ps_im[:])

            pos = 0
            for w in WIDTHS:
                sl = slice(pos, pos + w)
                pos += w
                nblk = w // MMF
                xt = xin.tile([P, COLS], BF16)
                ht = hin.tile([P, 4, COLS], FP8)
                nc.sync.dma_start(xt[:, 0:w], in_x[:, sl])
                nc.sync.dma_start(ht[:, :, 0:w], in_h[:, :, sl])
                lane_absorb(xt[0:1, 0:1])
                lane_absorb(ht[0:1, 0, 0:1])

                ot = outp.tile([P, 4, COLS], BF16)

                for c in range(HCHUNKS):
                    for b in range(nblk):
                        block(xt, ht, ot, c, b)
                # Merged store from ACT (HWDGE): queues behind ACT's own
                # copies; waits only on DVE's last oim copy. Never
                # blocks the load stream.
                nc.scalar.dma_start(out_t[:, :, sl], ot[:, :, 0:w])

    _split_multiwaits(nc)
    _cache["nc"] = nc
    return nc


def _split_multiwaits(nc):
    """walrus codegen allows exactly one semaphore wait per instruction.
    Move all-but-one wait of every multi-wait instruction onto single-wait
    NOP instructions spliced immediately before it on the same engine
    (engines execute their stream in order, so semantics are unchanged)."""
    k = 0
    for bb in nc.m.functions[0].blocks:
        new_list = []
        for ins in bb.instructions:
            si = ins.sync_info
            if si is not None and si.on_wait and len(si.on_wait) > 1:
                for w in si.on_wait[:-1]:
                    nop = mybir.InstNoOp(
                        name=f"WN-{k}", engine=ins.engine,
                        sync_info=mybir.SyncInfo(on_wait=[w], on_update=[]),
                    )
                    k += 1
                    new_list.append(nop)
                si.on_wait = [si.on_wait[-1]]
            new_list.append(ins)
        bb.instructions[:] = new_list


def kernel(inputs, h_re, h_im, nu_log, theta_log, B_real, B_img, gamma_log):
    global LAST_RESULTS
    inputs = np.asarray(inputs, dtype=np.float32)
    h_re = np.asarray(h_re, dtype=np.float32)
    h_im = np.asarray(h_im, dtype=np.float32)
    nu_log = np.asarray(nu_log, dtype=np.float32)
    theta_log = np.asarray(theta_log, dtype=np.float32)
    B_real = np.asarray(B_real, dtype=np.float32)
    B_img = np.asarray(B_img, dtype=np.float32)
    gamma_log = np.asarray(gamma_log, dtype=np.float32)

    # Tiny parameter math on host (matches the f32 reference computation).
    mag = np.exp(-np.exp(nu_log))          # (1, H)
    theta = np.exp(theta_log)              # (1, H)
    lam_re = (mag * np.cos(theta))[0]      # (H,)
    lam_im = (mag * np.sin(theta))[0]      # (H,)
    scale = np.exp(gamma_log).T            # (H, 1)
    w_re = (scale * B_real).T              # (IN_DIM, H)
    w_im = (scale * B_img).T               # (IN_DIM, H)

    consts = np.zeros((P, CONST_COLS), np.float32)
    consts[:, 0:HID] = w_re
    consts[:, HID:2 * HID] = w_im
    idx = np.arange(P)
    for c in range(HCHUNKS):
        lr = lam_re[c * P:(c + 1) * P]
        li = lam_im[c * P:(c + 1) * P]
        consts[idx, 2 * HID + c * P + idx] = lr
        consts[idx, 3 * HID + c * P + idx] = li
        consts[idx, 4 * HID + c * P + idx] = -li
    consts = consts.astype(NP_BF16)

    x_bf = inputs.astype(NP_BF16)
    hre_q = h_re.astype(NP_FP8)
    him_q = h_im.astype(NP_FP8)

    in_maps = []
    for core in range(N_CORES):
        sl = slice(core * S, (core + 1) * S)
        xT = np.ascontiguousarray(x_bf[sl].T)          # (128, S)
        hreT = hre_q[sl].T                             # (256, S)
        himT = him_q[sl].T
        hslab = np.empty((P, 4, S), NP_FP8)
        hslab[:, 0, :] = hreT[0:P]
        hslab[:, 1, :] = hreT[P:2 * P]
        hslab[:, 2, :] = himT[0:P]
        hslab[:, 3, :] = himT[P:2 * P]
        in_maps.append({"in_x": xT, "in_h": hslab, "consts": consts})

    nc = _build()
    res = run_bass_kernel_spmd(nc, in_maps, core_ids=list(range(N_CORES)))
    LAST_RESULTS = res

    out = np.empty((2, B_SZ, HID), np.float32)
    for core in range(N_CORES):
        sl = slice(core * S, (core + 1) * S)
        ob = res.results[core]["out_t"].astype(np.float32)  # (P, 4, S)
        for c in range(HCHUNKS):
            out[0, sl, c * P:(c + 1) * P] = ob[:, c, :].T
            out[1, sl, c * P:(c + 1) * P] = ob[:, 2 + c, :].T
    return out
